# revision 1
# baseline (speedup 1.0000x reference)
"""Trainium2 Bass kernel for nn_CustomLlamaAttention (B=2, S=2048, D=2048, H=16).

Sharding: batch*heads across 8 cores -> each core owns 2 heads x 2 batches.
Wq/Wk/Wv split column-wise (by head) per core; Wo split row-wise; each core
computes a partial [B,S,D] output which the host sums.

Per-core dataflow (everything transposed so no on-device transposes needed):
  QT/KT  [hd=128, S] = (Wq shard)^T-tiles (stationary) x X^T (moving)
  V      [S, hd] natural = X^T-tiles (stationary) x Wv^T (moving)
  RoPE on QT/KT in [d, s] layout (partition-shifted copy via DMA + 3 DVE ops)
  scoresT[sk, sq] = KT-tile (stationary) x QT (moving)      (no transpose!)
  expT = exp(scoresT / sqrt(hd)) on ScalarE (no max subtraction; logits ~ +-6)
  uoutT [hd, sq] += V-tile (stationary) x expT (moving)
  rowsums broadcast to 128 partitions via ones-matmul: ones x expT
  aT = uoutT * (1/rowsums)  -> partial += aT-tile (stationary) x Wo^T (moving)

Matmuls run as float32r (TF32): full PE rate. All matmul operands are produced
with float32r output dtype (the BIR verifier requires rounded producers);
host pre-rounds the DRAM-sourced operands to TF32 (RNE).
"""

import sys

for _p in ("/opt/trn_rl_repo", "/opt/trn_rl_repo/concourse"):
    if _p not in sys.path:
        sys.path.insert(0, _p)

import math

import numpy as np

# ---------------------------------------------------------------- config
N_CORES = 8
NUM_HEADS = 16
ROPE_BASE = 10000.0
HD = 128  # head dim

MM_DT = "float32r"  # "float32r" (TF32, fast) or "float32" (exact, 4x slower)

_CACHE = {}


def _full_cfg():
    return dict(B=2, S=2048, D=2048, NH=NUM_HEADS // N_CORES)


# ---------------------------------------------------------------- device program
def build_core_program(B, S, D, NH, mm_dt_name=None):
    """Build the single-core Bass program (identical on all 8 cores)."""
    import concourse.mybir as mybir
    from concourse import bacc
    from concourse.tile import TileContext

    if mm_dt_name is None:
        mm_dt_name = MM_DT
    f32 = mybir.dt.float32
    mdt = getattr(mybir.dt, mm_dt_name)

    def asf32(ap):
        return ap.bitcast(f32) if mdt != f32 else ap

    hd = HD
    half = hd // 2
    DQ = NH * hd           # per-core projection width (256)
    ET = D // 128          # contraction tiles over model dim
    SC = 256               # s-chunk width in projection phase
    NSC = S // SC
    SBK = SC // 128        # s-blocks per chunk (for V)
    SQT = min(512, S)      # attention sq tile width
    NSQ = S // SQT
    SKB = S // 128         # sk blocks
    SB = S // 128          # s blocks (Wo phase)
    EOW = min(512, D)      # output-proj tile width
    NEO = D // EOW
    RU = min(512, S)       # RoPE free-dim unit
    NRU = S // RU
    inv_sqrt_hd = 1.0 / math.sqrt(hd)

    nc = bacc.Bacc(trn_type="TRN2", target_bir_lowering=False)

    xt = nc.dram_tensor("xt", [B, ET, 128, S], mdt, kind="ExternalInput")
    wq = nc.dram_tensor("wq", [ET, 128, DQ], mdt, kind="ExternalInput")
    wk = nc.dram_tensor("wk", [ET, 128, DQ], mdt, kind="ExternalInput")
    wv = nc.dram_tensor("wv", [ET, 128, DQ], mdt, kind="ExternalInput")
    wo = nc.dram_tensor("wo", [NH, 128, D], mdt, kind="ExternalInput")
    cos = nc.dram_tensor("cos", [128, S], f32, kind="ExternalInput")
    sin = nc.dram_tensor("sin", [128, S], f32, kind="ExternalInput")  # sign-adjusted
    out = nc.dram_tensor("out", [B, SB, 128, D], f32, kind="ExternalOutput")

    Exp = mybir.ActivationFunctionType.Exp

    with TileContext(nc) as tc:
        with (
            tc.tile_pool(name="const", bufs=1) as const,
            tc.tile_pool(name="xtp", bufs=2) as xtp,
            tc.tile_pool(name="qk", bufs=1) as qk,
            tc.tile_pool(name="vp", bufs=2) as vp,
            tc.tile_pool(name="rp", bufs=1) as rp,
            tc.tile_pool(name="pp", bufs=3) as pp,
            tc.tile_pool(name="rr", bufs=1) as rr,
            tc.tile_pool(name="atp", bufs=1) as atp,
            tc.tile_pool(name="ow", bufs=1) as ow,
            tc.tile_pool(name="pj", bufs=2, space="PSUM") as pj,
            tc.tile_pool(name="sc", bufs=3, space="PSUM") as scp,
            tc.tile_pool(name="oc", bufs=2, space="PSUM") as ocp,
            tc.tile_pool(name="rc", bufs=1, space="PSUM") as rcp,
        ):
            # ---------- resident constants
            wq_sb = const.tile([128, ET, DQ], mdt, name="wq_sb")
            wk_sb = const.tile([128, ET, DQ], mdt, name="wk_sb")
            wv_sb = const.tile([128, ET, DQ], mdt, name="wv_sb")
            wo_sb = const.tile([128, NH, D], mdt, name="wo_sb")
            cos_sb = const.tile([128, S], f32, name="cos_sb")
            sin_sb = const.tile([128, S], f32, name="sin_sb")
            ones_sb = const.tile([128, 128], mdt, name="ones_sb")

            ETQ = max(1, ET // 4)

            def load_xt_chunk(b, c):
                csl = slice(c * SC, (c + 1) * SC)
                xt_sb = xtp.tile([128, ET, SC], mdt, tag="xt", name=f"xt_{b}_{c}")
                for q in range(0, ET, ETQ):
                    nc.sync.dma_start(
                        xt_sb[:, q : q + ETQ, :],
                        xt[b, q : q + ETQ, :, csl].rearrange("t p s -> p t s"),
                    )
                return xt_sb

            xt_next = load_xt_chunk(0, 0)
            for w_dram, w_tile in ((wq, wq_sb), (wk, wk_sb), (wv, wv_sb)):
                for q in range(0, ET, ETQ):
                    nc.scalar.dma_start(
                        w_tile[:, q : q + ETQ, :],
                        w_dram[q : q + ETQ].rearrange("t p d -> p t d"),
                    )
            nc.scalar.dma_start(cos_sb[:], cos[:])
            nc.scalar.dma_start(sin_sb[:], sin[:])
            ones_f32 = const.tile([128, 128], f32, name="ones_f32")
            nc.vector.memset(ones_f32[:], 1.0)
            nc.vector.tensor_copy(ones_sb[:], ones_f32[:])
            nc.scalar.dma_start(wo_sb[:], wo[:].rearrange("h p e -> p h e"))

            for b in range(B):
                # ---------- projections for batch b
                qt = [
                    qk.tile([128, S], mdt, tag=f"q{h}", name=f"qt{h}_{b}")
                    for h in range(NH)
                ]
                kt = [
                    qk.tile([128, S], mdt, tag=f"k{h}", name=f"kt{h}_{b}")
                    for h in range(NH)
                ]
                v_sb = vp.tile([128, SB, DQ], mdt, tag="v")

                def rope_unit(ten, u):
                    sl = slice(u * RU, (u + 1) * RU)
                    tcos = rp.tile([128, RU], f32, tag="rcos")
                    nc.vector.tensor_mul(tcos[:], asf32(ten[:, sl]), cos_sb[:, sl])
                    tsh = rp.tile([128, RU], mdt, tag="rsh")
                    nc.gpsimd.dma_start(tsh[0:half, :], ten[half:128, sl])
                    nc.gpsimd.dma_start(tsh[half:128, :], ten[0:half, sl])
                    nc.vector.tensor_mul(tsh[:], asf32(tsh[:]), sin_sb[:, sl])
                    nc.vector.tensor_add(ten[:, sl], tcos[:], asf32(tsh[:]))

                CPU = RU // SC  # chunks per rope unit
                for c in range(NSC):
                    csl = slice(c * SC, (c + 1) * SC)
                    xt_sb = xt_next
                    nxt = (b, c + 1) if c + 1 < NSC else (b + 1, 0)
                    if nxt[0] < B:
                        xt_next = load_xt_chunk(*nxt)
                    for h in range(NH):
                        for w_sb, dst in ((wq_sb, qt[h]), (wk_sb, kt[h])):
                            ps = pj.tile([128, SC], f32, tag="pj")
                            for t in range(ET):
                                nc.tensor.matmul(
                                    ps[:],
                                    w_sb[:, t, h * hd : (h + 1) * hd],
                                    xt_sb[:, t, :],
                                    start=(t == 0),
                                    stop=(t == ET - 1),
                                )
                            nc.scalar.copy(dst[:, csl], ps[:])
                    for s2 in range(SBK):
                        ps = pj.tile([128, DQ], f32, tag="pj", name="psv")
                        for t in range(ET):
                            nc.tensor.matmul(
                                ps[:],
                                xt_sb[:, t, s2 * 128 : (s2 + 1) * 128],
                                wv_sb[:, t, :],
                                start=(t == 0),
                                stop=(t == ET - 1),
                            )
                        nc.scalar.copy(v_sb[:, c * SBK + s2, :], ps[:])
                    if (c + 1) % CPU == 0:
                        u = (c + 1) // CPU - 1
                        for ten in (*qt, *kt):
                            rope_unit(ten, u)

                # ---------- attention per head
                at = [
                    atp.tile([128, S], mdt, tag=f"a{h}", name=f"at{h}_{b}")
                    for h in range(NH)
                ]
                def wo_block(sb_i):
                    ssl = slice(sb_i * 128, (sb_i + 1) * 128)
                    for eo in range(NEO):
                        eosl = slice(eo * EOW, (eo + 1) * EOW)
                        pw = scp.tile([128, EOW], f32, tag="sc", name="pw")
                        for a_t in range(NH):
                            nc.tensor.matmul(
                                pw[:],
                                at[a_t][:, ssl],
                                wo_sb[:, a_t, eosl],
                                start=(a_t == 0),
                                stop=(a_t == NH - 1),
                            )
                        osb = ow.tile([128, EOW], f32, tag="osb")
                        nc.vector.tensor_copy(osb[:], pw[:])
                        nc.sync.dma_start(out[b, sb_i, :, eosl], osb[:])

                SBQ = SQT // 128  # s-blocks per sq tile
                for qi in range(NSQ):
                    sq = slice(qi * SQT, (qi + 1) * SQT)
                    for h in range(NH):
                        po = ocp.tile([128, SQT], f32, tag="oc")
                        pr = rcp.tile([128, SQT], f32, tag="rc")

                        def score_exp(ki):
                            pscore = scp.tile(
                                [128, SQT], f32, tag="sc", name=f"psc{ki}"
                            )
                            nc.tensor.matmul(
                                pscore[:],
                                kt[h][:, ki * 128 : (ki + 1) * 128],
                                qt[h][:, sq],
                                start=True,
                                stop=True,
                            )
                            p_sb = pp.tile([128, SQT], mdt, tag="p", name=f"p{ki}")
                            nc.scalar.activation(
                                p_sb[:], pscore[:], Exp, scale=inv_sqrt_hd
                            )
                            return p_sb

                        p_next = score_exp(0)
                        for ki in range(SKB):
                            p_sb = p_next
                            if ki + 1 < SKB:
                                p_next = score_exp(ki + 1)
                            nc.tensor.matmul(
                                po[:],
                                v_sb[:, ki, h * hd : (h + 1) * hd],
                                p_sb[:],
                                start=(ki == 0),
                                stop=(ki == SKB - 1),
                            )
                            nc.tensor.matmul(
                                pr[:],
                                ones_sb[:],
                                p_sb[:],
                                start=(ki == 0),
                                stop=(ki == SKB - 1),
                            )
                        r_sb = rr.tile([128, SQT], f32, tag="r")
                        nc.vector.reciprocal_approx_fast(out=r_sb[:], in_=pr[:])
                        nc.vector.tensor_mul(at[h][:, sq], po[:], r_sb[:])
                    for sb_i in range(qi * SBQ, (qi + 1) * SBQ):
                        wo_block(sb_i)

    nc.compile()
    return nc


# ---------------------------------------------------------------- host helpers
def _round_tf32(x):
    """Round fp32 array to TF32-representable values (RNE on 10-bit mantissa)."""
    xi = np.ascontiguousarray(x, dtype=np.float32).view(np.uint32)
    lsb = (xi >> np.uint32(13)) & np.uint32(1)
    r = (xi + np.uint32(0x0FFF) + lsb) & np.uint32(0xFFFFE000)
    return r.view(np.float32)


def _rope_tables(S, dtype=np.float32):
    """cos table [128, S] and sign-adjusted sin table [128, S] in [d, s] layout."""
    inv_freq = 1.0 / (ROPE_BASE ** (np.arange(0, HD, 2, dtype=np.float32) / HD))
    t = np.arange(S, dtype=np.float32)
    freqs = np.outer(t, inv_freq)  # [S, half]
    cos = np.cos(freqs).T.astype(dtype)  # [half, S]
    sin = np.sin(freqs).T.astype(dtype)
    cosT = np.concatenate([cos, cos], axis=0)  # [128, S]
    sinT = np.concatenate([-sin, sin], axis=0)  # sign-adjusted for rotate_half
    return np.ascontiguousarray(cosT), np.ascontiguousarray(sinT)


def _prep_inputs(hidden_states, Wq, Wk, Wv, Wo, cfg, n_cores=N_CORES):
    """Build the per-core input dicts."""
    B, S, D, NH = cfg["B"], cfg["S"], cfg["D"], cfg["NH"]
    ET = D // 128
    DQ = NH * HD
    rnd = _round_tf32 if MM_DT == "float32r" else (
        lambda a: np.ascontiguousarray(a, dtype=np.float32)
    )

    x = np.asarray(hidden_states, dtype=np.float32)
    xt = rnd(np.ascontiguousarray(x.transpose(0, 2, 1))).reshape(B, ET, 128, S)
    cosT, sinT = _rope_tables(S)

    in_maps = []
    for c in range(n_cores):
        lo, hi = c * DQ, (c + 1) * DQ
        wq_c = rnd(np.asarray(Wq)[lo:hi, :].T).reshape(ET, 128, DQ)
        wk_c = rnd(np.asarray(Wk)[lo:hi, :].T).reshape(ET, 128, DQ)
        wv_c = rnd(np.asarray(Wv)[lo:hi, :].T).reshape(ET, 128, DQ)
        wo_c = rnd(np.asarray(Wo)[:, lo:hi].T).reshape(NH, 128, D)
        in_maps.append(
            {
                "xt": xt,
                "wq": wq_c,
                "wk": wk_c,
                "wv": wv_c,
                "wo": wo_c,
                "cos": cosT,
                "sin": sinT,
            }
        )
    return in_maps


def _gather(results, cfg):
    B, S, D = cfg["B"], cfg["S"], cfg["D"]
    acc = np.zeros((B, S, D), dtype=np.float64)
    for r in results:
        acc += r["out"].reshape(B, S, D).astype(np.float64)
    return acc.astype(np.float32)


# ---------------------------------------------------------------- entry point
def kernel(hidden_states, Wq, Wk, Wv, Wo):
    from concourse.bass_utils import run_bass_kernel_spmd

    cfg = _full_cfg()
    key = ("nc", cfg["B"], cfg["S"], cfg["D"], cfg["NH"], MM_DT)
    if key not in _CACHE:
        _CACHE[key] = build_core_program(cfg["B"], cfg["S"], cfg["D"], cfg["NH"])
    nc = _CACHE[key]

    in_maps = _prep_inputs(hidden_states, Wq, Wk, Wv, Wo, cfg)
    res = run_bass_kernel_spmd(nc, in_maps, core_ids=list(range(N_CORES)))
    return _gather(res.results, cfg)



# revision 3
# speedup vs baseline: 1.7001x; 1.7001x over previous
"""Trainium2 Bass kernel for nn_CustomLlamaAttention (B=2, S=2048, D=2048, H=16).

Sharding: batch*heads across 8 cores -> each core owns 2 heads x 2 batches.
Wq/Wk/Wv split column-wise (by head) per core; Wo split row-wise; each core
computes a partial [B,S,D] output (bf16) which the host sums in float64.

Per-core dataflow, all-bf16 operands (PSUM accumulation is fp32):
  QT/KT  [hd=128, S] = (Wq shard)^T-tiles (stationary) x X^T (moving)
  V      [S, hd] natural = X^T-tiles (stationary) x Wv^T (moving)
  RoPE: rotate_half is a signed 128x128 permutation matmul on the PE
        (prot = R^T q), then 3 DVE ops: q' = q*cos + prot*sin.
  scoresT[sk, sq] pairs: two 128-row score matmuls write one PSUM tile
        [128, 2, 512]; ONE wide ScalarE exp per pair -> p bf16 (amortizes
        the ACT engine's fixed ~352-cycle overhead so exp stays off the
        PE critical path).
  uoutT [hd, sq] += V-tile (stationary) x expT (moving)
  rowsums via ones-matmul; aT = uoutT * (1/rowsums) -> bf16
  partial += aT-tile (stationary) x Wo^T (moving), copied to bf16 and
        DMA'd out (copies alternate Scalar/Vector engines).

PSUM pools are phase-scoped per batch (proj 6KB / attn 16KB / wo 8KB).
"""

import sys

for _p in ("/opt/trn_rl_repo", "/opt/trn_rl_repo/concourse"):
    if _p not in sys.path:
        sys.path.insert(0, _p)

import math

import ml_dtypes
import numpy as np

# ---------------------------------------------------------------- config
N_CORES = 8
NUM_HEADS = 16
ROPE_BASE = 10000.0
HD = 128  # head dim

MM_DT = "bfloat16"  # kept for test.py compat; kernel is bf16-only

_CACHE = {}

BF16 = ml_dtypes.bfloat16


def _full_cfg():
    return dict(B=2, S=2048, D=2048, NH=NUM_HEADS // N_CORES)


# ---------------------------------------------------------------- device program
def build_core_program(B, S, D, NH, mm_dt_name=None):
    """Build the single-core Bass program (identical on all 8 cores)."""
    import concourse.mybir as mybir
    from concourse import bacc
    from concourse.tile import TileContext

    f32 = mybir.dt.float32
    bf = mybir.dt.bfloat16

    hd = HD
    DQ = NH * hd           # per-core projection width (256)
    ET = D // 128          # contraction tiles over model dim
    SC = 256               # s-chunk width in projection phase
    NSC = S // SC
    SBK = SC // 128        # s-blocks per chunk (for V)
    RU = min(512, S)       # RoPE unit width
    CPU = RU // SC         # chunks per rope unit
    SQT = min(512, S)      # attention sq tile width
    NSQ = S // SQT
    SKB = S // 128         # sk blocks
    NPAIR = SKB // 2
    SB = S // 128
    EOW = min(512, D)      # output-proj tile width
    NEO = D // EOW
    inv_sqrt_hd = 1.0 / math.sqrt(hd)

    nc = bacc.Bacc(trn_type="TRN2", target_bir_lowering=False)

    xt = nc.dram_tensor("xt", [B, ET, 128, S], bf, kind="ExternalInput")
    wq = nc.dram_tensor("wq", [ET, 128, DQ], bf, kind="ExternalInput")
    wk = nc.dram_tensor("wk", [ET, 128, DQ], bf, kind="ExternalInput")
    wv = nc.dram_tensor("wv", [ET, 128, DQ], bf, kind="ExternalInput")
    wo = nc.dram_tensor("wo", [NH, 128, D], bf, kind="ExternalInput")
    cos = nc.dram_tensor("cos", [128, S], bf, kind="ExternalInput")
    sin = nc.dram_tensor("sin", [128, S], bf, kind="ExternalInput")
    rotm = nc.dram_tensor("rotm", [128, 128], bf, kind="ExternalInput")
    out = nc.dram_tensor("out", [B, SB, 128, D], bf, kind="ExternalOutput")

    Exp = mybir.ActivationFunctionType.Exp

    with TileContext(nc) as tc:
        with (
            tc.tile_pool(name="const", bufs=1) as const,
            tc.tile_pool(name="xtp", bufs=2) as xtp,
            tc.tile_pool(name="qk", bufs=1) as qk,
            tc.tile_pool(name="vp", bufs=1) as vp,
            tc.tile_pool(name="qraw", bufs=2) as qrawp,
            tc.tile_pool(name="rtmp", bufs=2) as rtmp,
            tc.tile_pool(name="pp", bufs=3) as ppool,
            tc.tile_pool(name="rr", bufs=2) as rr,
            tc.tile_pool(name="atp", bufs=1) as atp,
            tc.tile_pool(name="ow", bufs=4) as ow,
        ):
            # ---------- resident constants
            wq_sb = const.tile([128, ET, DQ], bf, name="wq_sb")
            wk_sb = const.tile([128, ET, DQ], bf, name="wk_sb")
            wv_sb = const.tile([128, ET, DQ], bf, name="wv_sb")
            wo_sb = const.tile([128, NH, D], bf, name="wo_sb")
            cos_sb = const.tile([128, S], bf, name="cos_sb")
            sin_sb = const.tile([128, S], bf, name="sin_sb")
            rot_sb = const.tile([128, 128], bf, name="rot_sb")
            ones_sb = const.tile([128, 128], bf, name="ones_sb")

            ETQ = max(1, ET // 4)

            def load_xt_chunk(b, c):
                csl = slice(c * SC, (c + 1) * SC)
                xt_sb = xtp.tile([128, ET, SC], bf, tag="xt", name=f"xt_{b}_{c}")
                for qq in range(0, ET, ETQ):
                    nc.sync.dma_start(
                        xt_sb[:, qq : qq + ETQ, :],
                        xt[b, qq : qq + ETQ, :, csl].rearrange("t p s -> p t s"),
                    )
                return xt_sb

            xt_next = load_xt_chunk(0, 0)
            for w_dram, w_tile in ((wq, wq_sb), (wk, wk_sb), (wv, wv_sb)):
                for qq in range(0, ET, ETQ):
                    nc.scalar.dma_start(
                        w_tile[:, qq : qq + ETQ, :],
                        w_dram[qq : qq + ETQ].rearrange("t p d -> p t d"),
                    )
            nc.scalar.dma_start(cos_sb[:], cos[:])
            nc.scalar.dma_start(sin_sb[:], sin[:])
            nc.scalar.dma_start(rot_sb[:], rotm[:])
            ones_f32 = const.tile([128, 128], f32, name="ones_f32")
            nc.vector.memset(ones_f32[:], 1.0)
            nc.vector.tensor_copy(ones_sb[:], ones_f32[:])
            nc.scalar.dma_start(wo_sb[:], wo[:].rearrange("h p e -> p h e"))

            for b in range(B):
                # ---------- projections + RoPE for batch b
                qt = [
                    qk.tile([128, S], bf, tag=f"q{h}", name=f"qt{h}_{b}")
                    for h in range(NH)
                ]
                kt = [
                    qk.tile([128, S], bf, tag=f"k{h}", name=f"kt{h}_{b}")
                    for h in range(NH)
                ]
                v_sb = vp.tile([128, SB, DQ], bf, tag="v")

                with (
                    tc.tile_pool(name=f"pj{b}", bufs=2, space="PSUM") as pjp,
                    tc.tile_pool(name=f"rot{b}", bufs=2, space="PSUM") as rotp,
                ):
                    raw = {}  # (h, 0=q/1=k) -> staging tile for current unit
                    for c in range(NSC):
                        csl_u = slice((c % CPU) * SC, (c % CPU + 1) * SC)
                        xt_sb = xt_next
                        nxt = (b, c + 1) if c + 1 < NSC else (b + 1, 0)
                        if nxt[0] < B:
                            xt_next = load_xt_chunk(*nxt)
                        if c % CPU == 0:
                            for h in range(NH):
                                raw[(h, 0)] = qrawp.tile(
                                    [128, RU], bf, tag=f"qr{h}", name=f"qr{h}"
                                )
                                raw[(h, 1)] = qrawp.tile(
                                    [128, RU], bf, tag=f"kr{h}", name=f"kr{h}"
                                )
                        for h in range(NH):
                            for i, w_sb in enumerate((wq_sb, wk_sb)):
                                ps = pjp.tile([128, SC], f32, tag="pj")
                                for t in range(ET):
                                    nc.tensor.matmul(
                                        ps[:],
                                        w_sb[:, t, h * hd : (h + 1) * hd],
                                        xt_sb[:, t, :],
                                        start=(t == 0),
                                        stop=(t == ET - 1),
                                    )
                                nc.scalar.copy(raw[(h, i)][:, csl_u], ps[:])
                        for s2 in range(SBK):
                            psv = pjp.tile([128, DQ], f32, tag="pj", name="psv")
                            for t in range(ET):
                                nc.tensor.matmul(
                                    psv[:],
                                    xt_sb[:, t, s2 * 128 : (s2 + 1) * 128],
                                    wv_sb[:, t, :],
                                    start=(t == 0),
                                    stop=(t == ET - 1),
                                )
                            nc.scalar.copy(v_sb[:, c * SBK + s2, :], psv[:])
                        if (c + 1) % CPU == 0:
                            u = (c + 1) // CPU - 1
                            usl = slice(u * RU, (u + 1) * RU)
                            for h in range(NH):
                                for i, dst in ((0, qt[h]), (1, kt[h])):
                                    src = raw[(h, i)]
                                    prot = rotp.tile(
                                        [128, RU], f32, tag="rot"
                                    )
                                    nc.tensor.matmul(
                                        prot[:], rot_sb[:], src[:],
                                        start=True, stop=True,
                                    )
                                    tsin = rtmp.tile([128, RU], bf, tag="tsin")
                                    nc.vector.tensor_mul(
                                        tsin[:], prot[:], sin_sb[:, usl]
                                    )
                                    tcos = rtmp.tile([128, RU], bf, tag="tcos")
                                    nc.vector.tensor_mul(
                                        tcos[:], src[:], cos_sb[:, usl]
                                    )
                                    nc.vector.tensor_add(
                                        dst[:, usl], tcos[:], tsin[:]
                                    )

                # ---------- attention per head
                at = [
                    atp.tile([128, S], bf, tag=f"a{h}", name=f"at{h}_{b}")
                    for h in range(NH)
                ]
                with (
                    tc.tile_pool(name=f"sc{b}", bufs=2, space="PSUM") as scp,
                    tc.tile_pool(name=f"po{b}", bufs=2, space="PSUM") as pop,
                    tc.tile_pool(name=f"pr{b}", bufs=2, space="PSUM") as prp,
                ):
                    for qi in range(NSQ):
                        sq = slice(qi * SQT, (qi + 1) * SQT)
                        for h in range(NH):
                            po = pop.tile([128, SQT], f32, tag="oc")
                            pr = prp.tile([128, SQT], f32, tag="rc")

                            def score_pair(j):
                                sc_t = scp.tile(
                                    [128, 2, SQT], f32, tag="sc", name=f"sc{j}"
                                )
                                for i in range(2):
                                    ki = 2 * j + i
                                    nc.tensor.matmul(
                                        sc_t[:, i, :],
                                        kt[h][:, ki * 128 : (ki + 1) * 128],
                                        qt[h][:, sq],
                                        start=True,
                                        stop=True,
                                    )
                                p_sb = ppool.tile(
                                    [128, 2, SQT], bf, tag="p", name=f"p{j}"
                                )
                                nc.scalar.activation(
                                    p_sb[:], sc_t[:], Exp, scale=inv_sqrt_hd
                                )
                                return p_sb

                            p_next = score_pair(0)
                            for j in range(NPAIR):
                                p_sb = p_next
                                if j + 1 < NPAIR:
                                    p_next = score_pair(j + 1)
                                for i in range(2):
                                    ki = 2 * j + i
                                    nc.tensor.matmul(
                                        po[:],
                                        v_sb[:, ki, h * hd : (h + 1) * hd],
                                        p_sb[:, i, :],
                                        start=(ki == 0),
                                        stop=(ki == SKB - 1),
                                    )
                                    nc.tensor.matmul(
                                        pr[:],
                                        ones_sb[:],
                                        p_sb[:, i, :],
                                        start=(ki == 0),
                                        stop=(ki == SKB - 1),
                                    )
                            r_sb = rr.tile([128, SQT], f32, tag="r")
                            nc.vector.reciprocal_approx_fast(
                                out=r_sb[:], in_=pr[:]
                            )
                            nc.vector.tensor_mul(at[h][:, sq], po[:], r_sb[:])

                # ---------- output projection
                with tc.tile_pool(name=f"wo{b}", bufs=4, space="PSUM") as pwp:
                    for sb_i in range(SB):
                        ssl = slice(sb_i * 128, (sb_i + 1) * 128)
                        for eo in range(NEO):
                            eosl = slice(eo * EOW, (eo + 1) * EOW)
                            pw = pwp.tile([128, EOW], f32, tag="pw")
                            for a_t in range(NH):
                                nc.tensor.matmul(
                                    pw[:],
                                    at[a_t][:, ssl],
                                    wo_sb[:, a_t, eosl],
                                    start=(a_t == 0),
                                    stop=(a_t == NH - 1),
                                )
                            osb = ow.tile([128, EOW], bf, tag="osb")
                            if eo % 2 == 0:
                                nc.scalar.copy(osb[:], pw[:])
                            else:
                                nc.vector.tensor_copy(osb[:], pw[:])
                            (nc.sync if eo % 2 == 0 else nc.gpsimd).dma_start(
                                out[b, sb_i, :, eosl], osb[:]
                            )

    nc.compile()
    return nc


# ---------------------------------------------------------------- host helpers
def _rope_tables(S, dtype=BF16):
    """cos/sin tables [128, S] in [d, s] layout (plain sin; sign lives in
    the rotation matrix)."""
    inv_freq = 1.0 / (ROPE_BASE ** (np.arange(0, HD, 2, dtype=np.float32) / HD))
    t = np.arange(S, dtype=np.float32)
    freqs = np.outer(t, inv_freq)  # [S, half]
    cos = np.cos(freqs).T  # [half, S]
    sin = np.sin(freqs).T
    cosT = np.concatenate([cos, cos], axis=0).astype(dtype)  # [128, S]
    sinT = np.concatenate([sin, sin], axis=0).astype(dtype)
    return np.ascontiguousarray(cosT), np.ascontiguousarray(sinT)


def _rot_matrix(dtype=BF16):
    """Signed permutation R [128,128] (stationary layout) s.t.
    (R^T q)[i] = rotate_half(q)[i] for q in [d, s] layout."""
    half = HD // 2
    m = np.zeros((HD, HD), dtype=np.float32)
    for i in range(HD):
        m[(i + half) % HD, i] = -1.0 if i < half else 1.0
    return np.ascontiguousarray(m.astype(dtype))


def _prep_inputs(hidden_states, Wq, Wk, Wv, Wo, cfg, n_cores=N_CORES):
    """Build the per-core input dicts (all bf16)."""
    B, S, D, NH = cfg["B"], cfg["S"], cfg["D"], cfg["NH"]
    ET = D // 128
    DQ = NH * HD

    x = np.asarray(hidden_states, dtype=np.float32)
    xt = (
        np.ascontiguousarray(x.transpose(0, 2, 1))
        .astype(BF16)
        .reshape(B, ET, 128, S)
    )
    cosT, sinT = _rope_tables(S)
    rotmat = _rot_matrix()

    in_maps = []
    for c in range(n_cores):
        lo, hi = c * DQ, (c + 1) * DQ
        wq_c = np.ascontiguousarray(np.asarray(Wq)[lo:hi, :].T).astype(BF16)
        wk_c = np.ascontiguousarray(np.asarray(Wk)[lo:hi, :].T).astype(BF16)
        wv_c = np.ascontiguousarray(np.asarray(Wv)[lo:hi, :].T).astype(BF16)
        wo_c = np.ascontiguousarray(np.asarray(Wo)[:, lo:hi].T).astype(BF16)
        in_maps.append(
            {
                "xt": xt,
                "wq": wq_c.reshape(ET, 128, DQ),
                "wk": wk_c.reshape(ET, 128, DQ),
                "wv": wv_c.reshape(ET, 128, DQ),
                "wo": wo_c.reshape(NH, 128, D),
                "cos": cosT,
                "sin": sinT,
                "rotm": rotmat,
            }
        )
    return in_maps


def _gather(results, cfg):
    B, S, D = cfg["B"], cfg["S"], cfg["D"]
    acc = np.zeros((B, S, D), dtype=np.float64)
    for r in results:
        acc += np.asarray(r["out"]).astype(np.float64).reshape(B, S, D)
    return acc.astype(np.float32)


# ---------------------------------------------------------------- entry point
def kernel(hidden_states, Wq, Wk, Wv, Wo):
    from concourse.bass_utils import run_bass_kernel_spmd

    cfg = _full_cfg()
    key = ("nc", cfg["B"], cfg["S"], cfg["D"], cfg["NH"])
    if key not in _CACHE:
        _CACHE[key] = build_core_program(cfg["B"], cfg["S"], cfg["D"], cfg["NH"])
    nc = _CACHE[key]

    in_maps = _prep_inputs(hidden_states, Wq, Wk, Wv, Wo, cfg)
    res = run_bass_kernel_spmd(nc, in_maps, core_ids=list(range(N_CORES)))
    return _gather(res.results, cfg)


# revision 7
# speedup vs baseline: 1.8474x; 1.0867x over previous
"""Trainium2 Bass kernel for nn_CustomLlamaAttention (B=2, S=2048, D=2048, H=16).

Sharding: batch*heads across 8 cores -> each core owns 2 heads x 2 batches.
Wq/Wk/Wv split column-wise (by head) per core; Wo split row-wise; each core
computes a partial [B,S,D] output (bf16) which the host sums in float64.

Per-core dataflow, all-bf16 operands (PSUM accumulation is fp32):
  QT/KT  [hd=128, S] = (Wq shard)^T-tiles (stationary) x X^T (moving)
  V      [S, hd] natural = X^T-tiles (stationary) x Wv^T (moving)
  RoPE: rotate_half is a signed 128x128 permutation matmul on the PE
        (prot = R^T q), then 3 DVE ops: q' = q*cos + prot*sin.
  scoresT[sk, sq] pairs: two 128-row score matmuls write one PSUM tile
        [128, 2, 512]; ONE wide ScalarE exp per pair -> p bf16 (amortizes
        the ACT engine's fixed ~352-cycle overhead so exp stays off the
        PE critical path).
  uoutT [hd, sq] += V-tile (stationary) x expT (moving)
  rowsums via ones-matmul; aT = uoutT * (1/rowsums) -> bf16
  partial += aT-tile (stationary) x Wo^T (moving), copied to bf16 and
        DMA'd out (copies alternate Scalar/Vector engines).

PSUM pools are phase-scoped per batch (proj 6KB / attn 16KB / wo 8KB).
"""

import sys

for _p in ("/opt/trn_rl_repo", "/opt/trn_rl_repo/concourse"):
    if _p not in sys.path:
        sys.path.insert(0, _p)

import math

import ml_dtypes
import numpy as np

# ---------------------------------------------------------------- config
N_CORES = 8
NUM_HEADS = 16
ROPE_BASE = 10000.0
HD = 128  # head dim

MM_DT = "bfloat16"  # kept for test.py compat; kernel is bf16-only

_CACHE = {}

BF16 = ml_dtypes.bfloat16


def _full_cfg():
    return dict(B=2, S=2048, D=2048, NH=NUM_HEADS // N_CORES)


# ---------------------------------------------------------------- device program
def build_core_program(B, S, D, NH, mm_dt_name=None):
    """Build the single-core Bass program (identical on all 8 cores)."""
    import concourse.mybir as mybir
    from concourse import bacc
    from concourse.tile import TileContext

    f32 = mybir.dt.float32
    bf = mybir.dt.bfloat16

    hd = HD
    DQ = NH * hd           # per-core projection width (256)
    ET = D // 128          # contraction tiles over model dim
    SC = 256               # s-chunk width in projection phase
    NSC = S // SC
    SBK = SC // 128        # s-blocks per chunk (for V)
    RU = min(512, S)       # RoPE unit width
    CPU = RU // SC         # chunks per rope unit
    SQT = min(512, S)      # attention sq tile width
    NSQ = S // SQT
    SKB = S // 128         # sk blocks
    NPAIR = SKB // 2
    SB = S // 128
    EOW = min(512, D)      # output-proj tile width
    NEO = D // EOW
    inv_sqrt_hd = 1.0 / math.sqrt(hd)

    nc = bacc.Bacc(trn_type="TRN2", target_bir_lowering=False)

    xt = nc.dram_tensor("xt", [B, ET, 128, S], bf, kind="ExternalInput")
    wq = nc.dram_tensor("wq", [ET, 128, DQ], bf, kind="ExternalInput")
    wk = nc.dram_tensor("wk", [ET, 128, DQ], bf, kind="ExternalInput")
    wv = nc.dram_tensor("wv", [ET, 128, DQ], bf, kind="ExternalInput")
    wo = nc.dram_tensor("wo", [NH, 128, D], bf, kind="ExternalInput")
    cos = nc.dram_tensor("cos", [128, S], bf, kind="ExternalInput")
    sin = nc.dram_tensor("sin", [128, S], bf, kind="ExternalInput")
    rotm = nc.dram_tensor("rotm", [128, 128], bf, kind="ExternalInput")
    out = nc.dram_tensor("out", [B, SB, 128, D], bf, kind="ExternalOutput")

    Exp = mybir.ActivationFunctionType.Exp

    with TileContext(nc) as tc:
        with (
            tc.tile_pool(name="const", bufs=1) as const,
            tc.tile_pool(name="xtp", bufs=2) as xtp,
            tc.tile_pool(name="qk", bufs=1) as qk,
            tc.tile_pool(name="vp", bufs=1) as vp,
            tc.tile_pool(name="qraw", bufs=2) as qrawp,
            tc.tile_pool(name="rtmp", bufs=2) as rtmp,
            tc.tile_pool(name="pp", bufs=3) as ppool,
            tc.tile_pool(name="rr", bufs=2) as rr,
            tc.tile_pool(name="atp", bufs=1) as atp,
            tc.tile_pool(name="ow", bufs=4) as ow,
        ):
            # ---------- resident constants
            wq_sb = const.tile([128, ET, DQ], bf, name="wq_sb")
            wk_sb = const.tile([128, ET, DQ], bf, name="wk_sb")
            wv_sb = const.tile([128, ET, DQ], bf, name="wv_sb")
            wo_sb = const.tile([128, NH, D], bf, name="wo_sb")
            cos_sb = const.tile([128, S], bf, name="cos_sb")
            sin_sb = const.tile([128, S], bf, name="sin_sb")
            rot_sb = const.tile([128, 128], bf, name="rot_sb")
            ones_sb = const.tile([128, 128], bf, name="ones_sb")

            ETQ = max(1, ET // 4)

            def load_xt_chunk(b, c):
                csl = slice(c * SC, (c + 1) * SC)
                xt_sb = xtp.tile([128, ET, SC], bf, tag="xt", name=f"xt_{b}_{c}")
                for qq in range(0, ET, ETQ):
                    nc.sync.dma_start(
                        xt_sb[:, qq : qq + ETQ, :],
                        xt[b, qq : qq + ETQ, :, csl].rearrange("t p s -> p t s"),
                    )
                return xt_sb

            xt_next = load_xt_chunk(0, 0)
            for w_dram, w_tile, eng in (
                (wq, wq_sb, nc.scalar),
                (wk, wk_sb, nc.gpsimd),
                (wv, wv_sb, nc.gpsimd),
            ):
                for qq in range(0, ET, ETQ):
                    eng.dma_start(
                        w_tile[:, qq : qq + ETQ, :],
                        w_dram[qq : qq + ETQ].rearrange("t p d -> p t d"),
                    )
            nc.scalar.dma_start(cos_sb[:], cos[:])
            nc.scalar.dma_start(sin_sb[:], sin[:])
            nc.scalar.dma_start(rot_sb[:], rotm[:])
            ones_f32 = const.tile([128, 128], f32, name="ones_f32")
            nc.vector.memset(ones_f32[:], 1.0)
            nc.vector.tensor_copy(ones_sb[:], ones_f32[:])
            nc.scalar.dma_start(wo_sb[:], wo[:].rearrange("h p e -> p h e"))

            for b in range(B):
                # ---------- projections + RoPE for batch b
                qt = [
                    qk.tile([128, S], bf, tag=f"q{h}", name=f"qt{h}_{b}")
                    for h in range(NH)
                ]
                kt = [
                    qk.tile([128, S], bf, tag=f"k{h}", name=f"kt{h}_{b}")
                    for h in range(NH)
                ]
                v_sb = vp.tile([128, SB, DQ], bf, tag="v")

                with (
                    tc.tile_pool(name=f"pj{b}", bufs=2, space="PSUM") as pjp,
                    tc.tile_pool(name=f"rot{b}", bufs=2, space="PSUM") as rotp,
                ):
                    raw = {}  # (h, 0=q/1=k) -> staging tile for current unit
                    for c in range(NSC):
                        csl_u = slice((c % CPU) * SC, (c % CPU + 1) * SC)
                        xt_sb = xt_next
                        nxt = (b, c + 1) if c + 1 < NSC else (b + 1, 0)
                        if nxt[0] < B:
                            xt_next = load_xt_chunk(*nxt)
                        if c % CPU == 0:
                            for h in range(NH):
                                raw[(h, 0)] = qrawp.tile(
                                    [128, RU], bf, tag=f"qr{h}", name=f"qr{h}"
                                )
                                raw[(h, 1)] = qrawp.tile(
                                    [128, RU], bf, tag=f"kr{h}", name=f"kr{h}"
                                )
                        for h in range(NH):
                            for i, w_sb in enumerate((wq_sb, wk_sb)):
                                ps = pjp.tile([128, SC], f32, tag="pj")
                                for t in range(ET):
                                    nc.tensor.matmul(
                                        ps[:],
                                        w_sb[:, t, h * hd : (h + 1) * hd],
                                        xt_sb[:, t, :],
                                        start=(t == 0),
                                        stop=(t == ET - 1),
                                    )
                                nc.scalar.copy(raw[(h, i)][:, csl_u], ps[:])
                        for s2 in range(SBK):
                            psv = pjp.tile([128, DQ], f32, tag="pj", name="psv")
                            for t in range(ET):
                                nc.tensor.matmul(
                                    psv[:],
                                    xt_sb[:, t, s2 * 128 : (s2 + 1) * 128],
                                    wv_sb[:, t, :],
                                    start=(t == 0),
                                    stop=(t == ET - 1),
                                )
                            nc.scalar.copy(v_sb[:, c * SBK + s2, :], psv[:])
                        if (c + 1) % CPU == 0:
                            u = (c + 1) // CPU - 1
                            usl = slice(u * RU, (u + 1) * RU)
                            for h in range(NH):
                                for i, dst in ((0, qt[h]), (1, kt[h])):
                                    src = raw[(h, i)]
                                    prot = rotp.tile(
                                        [128, RU], f32, tag="rot"
                                    )
                                    nc.tensor.matmul(
                                        prot[:], rot_sb[:], src[:],
                                        start=True, stop=True,
                                    )
                                    tsin = rtmp.tile([128, RU], bf, tag="tsin")
                                    nc.vector.tensor_mul(
                                        tsin[:], prot[:], sin_sb[:, usl]
                                    )
                                    tcos = rtmp.tile([128, RU], bf, tag="tcos")
                                    nc.vector.tensor_mul(
                                        tcos[:], src[:], cos_sb[:, usl]
                                    )
                                    nc.vector.tensor_add(
                                        dst[:, usl], tcos[:], tsin[:]
                                    )

                # ---------- attention per head
                at = [
                    atp.tile([128, S], bf, tag=f"a{h}", name=f"at{h}_{b}")
                    for h in range(NH)
                ]
                with (
                    tc.tile_pool(name=f"sc{b}", bufs=2, space="PSUM") as scp,
                    tc.tile_pool(name=f"po{b}", bufs=2, space="PSUM") as pop,
                    tc.tile_pool(name=f"pr{b}", bufs=2, space="PSUM") as prp,
                ):
                    # flat software pipeline over (qi, h, pair) so the exp
                    # latency is only exposed once per batch, not per (qi, h)
                    tasks = [(qi, h) for qi in range(NSQ) for h in range(NH)]
                    allpairs = [
                        (ti, j)
                        for ti in range(len(tasks))
                        for j in range(NPAIR)
                    ]

                    def score_pair(ti, j):
                        qi, h = tasks[ti]
                        sq = slice(qi * SQT, (qi + 1) * SQT)
                        sc_t = scp.tile(
                            [128, 2, SQT], f32, tag="sc", name=f"sc{ti}_{j}"
                        )
                        for i in range(2):
                            ki = 2 * j + i
                            nc.tensor.matmul(
                                sc_t[:, i, :],
                                kt[h][:, ki * 128 : (ki + 1) * 128],
                                qt[h][:, sq],
                                start=True,
                                stop=True,
                            )
                        p_sb = ppool.tile(
                            [128, 2, SQT], bf, tag="p", name=f"p{ti}_{j}"
                        )
                        nc.scalar.activation(
                            p_sb[:], sc_t[:], Exp, scale=inv_sqrt_hd
                        )
                        return p_sb

                    po = pr = None
                    p_next = score_pair(*allpairs[0])
                    for idx, (ti, j) in enumerate(allpairs):
                        qi, h = tasks[ti]
                        sq = slice(qi * SQT, (qi + 1) * SQT)
                        p_sb = p_next
                        if idx + 1 < len(allpairs):
                            p_next = score_pair(*allpairs[idx + 1])
                        if j == 0:
                            po = pop.tile([128, SQT], f32, tag="oc")
                            pr = prp.tile([128, SQT], f32, tag="rc")
                        for i in range(2):
                            ki = 2 * j + i
                            nc.tensor.matmul(
                                po[:],
                                v_sb[:, ki, h * hd : (h + 1) * hd],
                                p_sb[:, i, :],
                                start=(ki == 0),
                                stop=(ki == SKB - 1),
                            )
                            nc.tensor.matmul(
                                pr[:],
                                ones_sb[:],
                                p_sb[:, i, :],
                                start=(ki == 0),
                                stop=(ki == SKB - 1),
                            )
                        if j == NPAIR - 1:
                            r_sb = rr.tile([128, SQT], f32, tag="r")
                            nc.vector.reciprocal_approx_fast(
                                out=r_sb[:], in_=pr[:]
                            )
                            nc.vector.tensor_mul(
                                at[h][:, sq], po[:], r_sb[:]
                            )

                # ---------- output projection (full-D rows: 8 matmuls into
                # one [128, D] PSUM tile, then ONE wide copy + ONE DMA)
                with tc.tile_pool(name=f"wo{b}", bufs=2, space="PSUM") as pwp:
                    for sb_i in range(SB):
                        ssl = slice(sb_i * 128, (sb_i + 1) * 128)
                        pw = pwp.tile([128, D], f32, tag="pw")
                        for eo in range(NEO):
                            eosl = slice(eo * EOW, (eo + 1) * EOW)
                            for a_t in range(NH):
                                nc.tensor.matmul(
                                    pw[:, eosl],
                                    at[a_t][:, ssl],
                                    wo_sb[:, a_t, eosl],
                                    start=(a_t == 0),
                                    stop=(a_t == NH - 1),
                                )
                        osb = ow.tile([128, D], bf, tag="osb")
                        if sb_i % 2 == 0:
                            nc.scalar.copy(osb[:], pw[:])
                        else:
                            nc.vector.tensor_copy(osb[:], pw[:])
                        (nc.sync if sb_i % 2 == 0 else nc.gpsimd).dma_start(
                            out[b, sb_i, :, :], osb[:]
                        )

    nc.compile()
    return nc


# ---------------------------------------------------------------- host helpers
def _rope_tables(S, dtype=BF16):
    """cos/sin tables [128, S] in [d, s] layout (plain sin; sign lives in
    the rotation matrix)."""
    inv_freq = 1.0 / (ROPE_BASE ** (np.arange(0, HD, 2, dtype=np.float32) / HD))
    t = np.arange(S, dtype=np.float32)
    freqs = np.outer(t, inv_freq)  # [S, half]
    cos = np.cos(freqs).T  # [half, S]
    sin = np.sin(freqs).T
    cosT = np.concatenate([cos, cos], axis=0).astype(dtype)  # [128, S]
    sinT = np.concatenate([sin, sin], axis=0).astype(dtype)
    return np.ascontiguousarray(cosT), np.ascontiguousarray(sinT)


def _rot_matrix(dtype=BF16):
    """Signed permutation R [128,128] (stationary layout) s.t.
    (R^T q)[i] = rotate_half(q)[i] for q in [d, s] layout."""
    half = HD // 2
    m = np.zeros((HD, HD), dtype=np.float32)
    for i in range(HD):
        m[(i + half) % HD, i] = -1.0 if i < half else 1.0
    return np.ascontiguousarray(m.astype(dtype))


def _prep_inputs(hidden_states, Wq, Wk, Wv, Wo, cfg, n_cores=N_CORES):
    """Build the per-core input dicts (all bf16)."""
    B, S, D, NH = cfg["B"], cfg["S"], cfg["D"], cfg["NH"]
    ET = D // 128
    DQ = NH * HD

    x = np.asarray(hidden_states, dtype=np.float32)
    xt = (
        np.ascontiguousarray(x.transpose(0, 2, 1))
        .astype(BF16)
        .reshape(B, ET, 128, S)
    )
    cosT, sinT = _rope_tables(S)
    rotmat = _rot_matrix()

    in_maps = []
    for c in range(n_cores):
        lo, hi = c * DQ, (c + 1) * DQ
        wq_c = np.ascontiguousarray(np.asarray(Wq)[lo:hi, :].T).astype(BF16)
        wk_c = np.ascontiguousarray(np.asarray(Wk)[lo:hi, :].T).astype(BF16)
        wv_c = np.ascontiguousarray(np.asarray(Wv)[lo:hi, :].T).astype(BF16)
        wo_c = np.ascontiguousarray(np.asarray(Wo)[:, lo:hi].T).astype(BF16)
        in_maps.append(
            {
                "xt": xt,
                "wq": wq_c.reshape(ET, 128, DQ),
                "wk": wk_c.reshape(ET, 128, DQ),
                "wv": wv_c.reshape(ET, 128, DQ),
                "wo": wo_c.reshape(NH, 128, D),
                "cos": cosT,
                "sin": sinT,
                "rotm": rotmat,
            }
        )
    return in_maps


def _gather(results, cfg):
    B, S, D = cfg["B"], cfg["S"], cfg["D"]
    acc = np.zeros((B, S, D), dtype=np.float64)
    for r in results:
        acc += np.asarray(r["out"]).astype(np.float64).reshape(B, S, D)
    return acc.astype(np.float32)


# ---------------------------------------------------------------- entry point
def kernel(hidden_states, Wq, Wk, Wv, Wo):
    from concourse.bass_utils import run_bass_kernel_spmd

    cfg = _full_cfg()
    key = ("nc", cfg["B"], cfg["S"], cfg["D"], cfg["NH"])
    if key not in _CACHE:
        _CACHE[key] = build_core_program(cfg["B"], cfg["S"], cfg["D"], cfg["NH"])
    nc = _CACHE[key]

    in_maps = _prep_inputs(hidden_states, Wq, Wk, Wv, Wo, cfg)
    res = run_bass_kernel_spmd(nc, in_maps, core_ids=list(range(N_CORES)))
    return _gather(res.results, cfg)


# revision 15
# speedup vs baseline: 1.8542x; 1.0037x over previous
"""Trainium2 Bass kernel for nn_CustomLlamaAttention (B=2, S=2048, D=2048, H=16).

Sharding: batch*heads across 8 cores -> each core owns 2 heads x 2 batches.
Wq/Wk/Wv split column-wise (by head) per core; Wo split row-wise; each core
computes a partial [B,S,D] output (bf16) which the host sums in float64.

Per-core dataflow, all-bf16 operands (PSUM accumulation is fp32):
  QT/KT  [hd=128, S] = (Wq shard)^T-tiles (stationary) x X^T (moving)
  V      [S, hd] natural = X^T-tiles (stationary) x Wv^T (moving)
  RoPE: rotate_half is a signed 128x128 permutation matmul on the PE
        (prot = R^T q), then 3 DVE ops: q' = q*cos + prot*sin.
  scoresT[sk, sq] pairs: two 128-row score matmuls write one PSUM tile
        [128, 2, 512]; ONE wide ScalarE exp per pair -> p bf16 (amortizes
        the ACT engine's fixed ~352-cycle overhead so exp stays off the
        PE critical path).
  uoutT [hd, sq] += V-tile (stationary) x expT (moving)
  rowsums via ones-matmul; aT = uoutT * (1/rowsums) -> bf16
  partial += aT-tile (stationary) x Wo^T (moving), copied to bf16 and
        DMA'd out (copies alternate Scalar/Vector engines).

PSUM pools are phase-scoped per batch (proj 6KB / attn 16KB / wo 8KB).
"""

import sys

for _p in ("/opt/trn_rl_repo", "/opt/trn_rl_repo/concourse"):
    if _p not in sys.path:
        sys.path.insert(0, _p)

import math

import ml_dtypes
import numpy as np

# ---------------------------------------------------------------- config
N_CORES = 8
NUM_HEADS = 16
ROPE_BASE = 10000.0
HD = 128  # head dim

MM_DT = "bfloat16"  # kept for test.py compat; kernel is bf16-only

_CACHE = {}

BF16 = ml_dtypes.bfloat16


def _full_cfg():
    return dict(B=2, S=2048, D=2048, NH=NUM_HEADS // N_CORES)


# ---------------------------------------------------------------- device program
def build_core_program(B, S, D, NH, mm_dt_name=None):
    """Build the single-core Bass program (identical on all 8 cores)."""
    import concourse.mybir as mybir
    from concourse import bacc
    from concourse.tile import TileContext

    f32 = mybir.dt.float32
    bf = mybir.dt.bfloat16

    hd = HD
    DQ = NH * hd           # per-core projection width (256)
    ET = D // 128          # contraction tiles over model dim
    SC = 256               # s-chunk width in projection phase
    NSC = S // SC
    SBK = SC // 128        # s-blocks per chunk (for V)
    RU = min(512, S)       # RoPE unit width
    CPU = RU // SC         # chunks per rope unit
    SQT = min(512, S)      # attention sq tile width
    NSQ = S // SQT
    SKB = S // 128         # sk blocks
    NPAIR = SKB // 2
    SB = S // 128
    EOW = min(512, D)      # output-proj tile width
    NEO = D // EOW
    inv_sqrt_hd = 1.0 / math.sqrt(hd)

    nc = bacc.Bacc(trn_type="TRN2", target_bir_lowering=False)

    xt = nc.dram_tensor("xt", [B, ET, 128, S], bf, kind="ExternalInput")
    wq = nc.dram_tensor("wq", [ET, 128, DQ], bf, kind="ExternalInput")
    wk = nc.dram_tensor("wk", [ET, 128, DQ], bf, kind="ExternalInput")
    wv = nc.dram_tensor("wv", [ET, 128, DQ], bf, kind="ExternalInput")
    wo = nc.dram_tensor("wo", [NH, 128, D], bf, kind="ExternalInput")
    cos = nc.dram_tensor("cos", [128, S], bf, kind="ExternalInput")
    sin = nc.dram_tensor("sin", [128, S], bf, kind="ExternalInput")
    rotm = nc.dram_tensor("rotm", [128, 128], bf, kind="ExternalInput")
    out = nc.dram_tensor("out", [B, SB, 128, D], bf, kind="ExternalOutput")

    Exp = mybir.ActivationFunctionType.Exp

    with TileContext(nc) as tc:
        with (
            tc.tile_pool(name="const", bufs=1) as const,
            tc.tile_pool(name="xtp", bufs=2) as xtp,
            tc.tile_pool(name="qk", bufs=1) as qk,
            tc.tile_pool(name="vp", bufs=1) as vp,
            tc.tile_pool(name="qraw", bufs=2) as qrawp,
            tc.tile_pool(name="rtmp", bufs=2) as rtmp,
            tc.tile_pool(name="pp", bufs=3) as ppool,
            tc.tile_pool(name="rr", bufs=2) as rr,
            tc.tile_pool(name="atp", bufs=1) as atp,
            tc.tile_pool(name="ow", bufs=4) as ow,
        ):
            # ---------- resident constants
            # Weights and xt chunks are split into ETQ-row groups with one
            # tile per group, so the first matmuls only wait on the first
            # group's DMA instead of the whole tensor (tile-level deps).
            ETQ = max(1, ET // 4)
            NG = ET // ETQ

            def wgroup_tiles(nm):
                return [
                    const.tile([128, ETQ, DQ], bf, name=f"{nm}_sb{g}")
                    for g in range(NG)
                ]

            wq_sb = wgroup_tiles("wq")
            wk_sb = wgroup_tiles("wk")
            wv_sb = wgroup_tiles("wv")
            wo_sb = const.tile([128, NH, D], bf, name="wo_sb")
            cos_sb = const.tile([128, S], bf, name="cos_sb")
            sin_sb = const.tile([128, S], bf, name="sin_sb")
            rot_sb = const.tile([128, 128], bf, name="rot_sb")
            ones_sb = const.tile([128, 128], bf, name="ones_sb")

            def load_xt_chunk(b, c):
                csl = slice(c * SC, (c + 1) * SC)
                tiles = []
                for g in range(NG):
                    t_sb = xtp.tile(
                        [128, ETQ, SC], bf, tag=f"xt{g}", name=f"xt_{b}_{c}_{g}"
                    )
                    nc.sync.dma_start(
                        t_sb[:],
                        xt[b, g * ETQ : (g + 1) * ETQ, :, csl].rearrange(
                            "t p s -> p t s"
                        ),
                    )
                    tiles.append(t_sb)
                return tiles

            xt_next = load_xt_chunk(0, 0)
            for g in range(NG):
                for w_dram, w_tiles in ((wq, wq_sb), (wk, wk_sb), (wv, wv_sb)):
                    nc.scalar.dma_start(
                        w_tiles[g][:],
                        w_dram[g * ETQ : (g + 1) * ETQ].rearrange(
                            "t p d -> p t d"
                        ),
                    )
            nc.scalar.dma_start(cos_sb[:], cos[:])
            nc.scalar.dma_start(sin_sb[:], sin[:])
            nc.scalar.dma_start(rot_sb[:], rotm[:])
            ones_f32 = const.tile([128, 128], f32, name="ones_f32")
            nc.vector.memset(ones_f32[:], 1.0)
            nc.vector.tensor_copy(ones_sb[:], ones_f32[:])
            nc.scalar.dma_start(wo_sb[:], wo[:].rearrange("h p e -> p h e"))

            HW_ = D // 2
            EOW2 = min(EOW, HW_)
            NEO2 = HW_ // EOW2

            def emit_wo(bb, at_list, pwp):
                # output projection for batch bb: half-D rows ([128, D/2]
                # PSUM accum -> ONE wide copy -> ONE DMA per half; copies
                # alternate Scalar/Vector, DMAs alternate sync/gpsimd
                for sb_i in range(SB):
                    ssl = slice(sb_i * 128, (sb_i + 1) * 128)
                    for half in range(2):
                        pw = pwp.tile([128, HW_], f32, tag="pw")
                        for eo in range(NEO2):
                            eosl = slice(
                                half * HW_ + eo * EOW2,
                                half * HW_ + (eo + 1) * EOW2,
                            )
                            for a_t in range(NH):
                                nc.tensor.matmul(
                                    pw[:, eo * EOW2 : (eo + 1) * EOW2],
                                    at_list[a_t][:, ssl],
                                    wo_sb[:, a_t, eosl],
                                    start=(a_t == 0),
                                    stop=(a_t == NH - 1),
                                )
                        osb = ow.tile([128, HW_], bf, tag="osb")
                        if half == 0:
                            nc.scalar.copy(osb[:], pw[:])
                        else:
                            nc.vector.tensor_copy(osb[:], pw[:])
                        (nc.sync if half == 0 else nc.gpsimd).dma_start(
                            out[bb, sb_i, :, half * HW_ : (half + 1) * HW_],
                            osb[:],
                        )

            at_prev = None
            for b in range(B):
                # ---------- projections + RoPE for batch b
                qt = [
                    qk.tile([128, S], bf, tag=f"q{h}", name=f"qt{h}_{b}")
                    for h in range(NH)
                ]
                kt = [
                    qk.tile([128, S], bf, tag=f"k{h}", name=f"kt{h}_{b}")
                    for h in range(NH)
                ]
                # V split into quarters so early AV matmuls don't wait on
                # the whole projection phase (tile-level deps)
                SBV = max(1, SB // 4)
                v_sb = [
                    vp.tile([128, SBV, DQ], bf, tag=f"v{g}", name=f"v{g}_{b}")
                    for g in range(SB // SBV)
                ]

                with (
                    tc.tile_pool(name=f"pj{b}", bufs=2, space="PSUM") as pjp,
                    tc.tile_pool(name=f"rot{b}", bufs=2, space="PSUM") as rotp,
                    tc.tile_pool(name=f"wo{b}", bufs=2, space="PSUM") as pwp,
                ):
                    # previous batch's output projection first: shares this
                    # pool scope so no PSUM pool boundary stalls the PE
                    if at_prev is not None:
                        emit_wo(b - 1, at_prev, pwp)
                    raw = {}  # (h, 0=q/1=k) -> staging tile for current unit
                    for c in range(NSC):
                        csl_u = slice((c % CPU) * SC, (c % CPU + 1) * SC)
                        xt_sb = xt_next
                        nxt = (b, c + 1) if c + 1 < NSC else (b + 1, 0)
                        if nxt[0] < B:
                            xt_next = load_xt_chunk(*nxt)
                        if c % CPU == 0:
                            for h in range(NH):
                                raw[(h, 0)] = qrawp.tile(
                                    [128, RU], bf, tag=f"qr{h}", name=f"qr{h}"
                                )
                                raw[(h, 1)] = qrawp.tile(
                                    [128, RU], bf, tag=f"kr{h}", name=f"kr{h}"
                                )
                        for h in range(NH):
                            for i, w_sb in enumerate((wq_sb, wk_sb)):
                                ps = pjp.tile([128, SC], f32, tag="pj")
                                for t in range(ET):
                                    nc.tensor.matmul(
                                        ps[:],
                                        w_sb[t // ETQ][
                                            :, t % ETQ, h * hd : (h + 1) * hd
                                        ],
                                        xt_sb[t // ETQ][:, t % ETQ, :],
                                        start=(t == 0),
                                        stop=(t == ET - 1),
                                    )
                                nc.scalar.copy(raw[(h, i)][:, csl_u], ps[:])
                        for s2 in range(SBK):
                            psv = pjp.tile([128, DQ], f32, tag="pj", name="psv")
                            for t in range(ET):
                                nc.tensor.matmul(
                                    psv[:],
                                    xt_sb[t // ETQ][
                                        :, t % ETQ, s2 * 128 : (s2 + 1) * 128
                                    ],
                                    wv_sb[t // ETQ][:, t % ETQ, :],
                                    start=(t == 0),
                                    stop=(t == ET - 1),
                                )
                            cb = c * SBK + s2
                            nc.scalar.copy(
                                v_sb[cb // SBV][:, cb % SBV, :], psv[:]
                            )
                        if (c + 1) % CPU == 0:
                            u = (c + 1) // CPU - 1
                            usl = slice(u * RU, (u + 1) * RU)
                            for h in range(NH):
                                for i, dst in ((0, qt[h]), (1, kt[h])):
                                    src = raw[(h, i)]
                                    prot = rotp.tile(
                                        [128, RU], f32, tag="rot"
                                    )
                                    nc.tensor.matmul(
                                        prot[:], rot_sb[:], src[:],
                                        start=True, stop=True,
                                    )
                                    tsin = rtmp.tile([128, RU], bf, tag="tsin")
                                    nc.vector.tensor_mul(
                                        tsin[:], prot[:], sin_sb[:, usl]
                                    )
                                    tcos = rtmp.tile([128, RU], bf, tag="tcos")
                                    nc.vector.tensor_mul(
                                        tcos[:], src[:], cos_sb[:, usl]
                                    )
                                    nc.vector.tensor_add(
                                        dst[:, usl], tcos[:], tsin[:]
                                    )

                # ---------- attention per head
                at = [
                    atp.tile([128, S], bf, tag=f"a{h}", name=f"at{h}_{b}")
                    for h in range(NH)
                ]
                with (
                    tc.tile_pool(name=f"sc{b}", bufs=2, space="PSUM") as scp,
                    tc.tile_pool(name=f"po{b}", bufs=2, space="PSUM") as pop,
                    tc.tile_pool(name=f"pr{b}", bufs=2, space="PSUM") as prp,
                ):
                    # flat software pipeline over (qi, h, pair) so the exp
                    # latency is only exposed once per batch, not per (qi, h)
                    tasks = [(qi, h) for qi in range(NSQ) for h in range(NH)]
                    allpairs = [
                        (ti, j)
                        for ti in range(len(tasks))
                        for j in range(NPAIR)
                    ]

                    def score_pair(ti, j):
                        qi, h = tasks[ti]
                        sq = slice(qi * SQT, (qi + 1) * SQT)
                        sc_t = scp.tile(
                            [128, 2, SQT], f32, tag="sc", name=f"sc{ti}_{j}"
                        )
                        for i in range(2):
                            ki = 2 * j + i
                            nc.tensor.matmul(
                                sc_t[:, i, :],
                                kt[h][:, ki * 128 : (ki + 1) * 128],
                                qt[h][:, sq],
                                start=True,
                                stop=True,
                            )
                        p_sb = ppool.tile(
                            [128, 2, SQT], bf, tag="p", name=f"p{ti}_{j}"
                        )
                        nc.scalar.activation(
                            p_sb[:], sc_t[:], Exp, scale=inv_sqrt_hd
                        )
                        return p_sb

                    po = pr = None
                    p_next = score_pair(*allpairs[0])
                    for idx, (ti, j) in enumerate(allpairs):
                        qi, h = tasks[ti]
                        sq = slice(qi * SQT, (qi + 1) * SQT)
                        p_sb = p_next
                        if idx + 1 < len(allpairs):
                            p_next = score_pair(*allpairs[idx + 1])
                        if j == 0:
                            po = pop.tile([128, SQT], f32, tag="oc")
                            pr = prp.tile([128, SQT], f32, tag="rc")
                        for i in range(2):
                            ki = 2 * j + i
                            nc.tensor.matmul(
                                po[:],
                                v_sb[ki // SBV][
                                    :, ki % SBV, h * hd : (h + 1) * hd
                                ],
                                p_sb[:, i, :],
                                start=(ki == 0),
                                stop=(ki == SKB - 1),
                            )
                            nc.tensor.matmul(
                                pr[:],
                                ones_sb[:],
                                p_sb[:, i, :],
                                start=(ki == 0),
                                stop=(ki == SKB - 1),
                            )
                        if j == NPAIR - 1:
                            r_sb = rr.tile([128, SQT], f32, tag="r")
                            nc.vector.reciprocal_approx_fast(
                                out=r_sb[:], in_=pr[:]
                            )
                            nc.vector.tensor_mul(
                                at[h][:, sq], po[:], r_sb[:]
                            )

                at_prev = at

            # final batch's output projection
            with tc.tile_pool(name="wolast", bufs=2, space="PSUM") as pwp:
                emit_wo(B - 1, at_prev, pwp)

    nc.compile()
    return nc


# ---------------------------------------------------------------- host helpers
def _rope_tables(S, dtype=BF16):
    """cos/sin tables [128, S] in [d, s] layout (plain sin; sign lives in
    the rotation matrix)."""
    inv_freq = 1.0 / (ROPE_BASE ** (np.arange(0, HD, 2, dtype=np.float32) / HD))
    t = np.arange(S, dtype=np.float32)
    freqs = np.outer(t, inv_freq)  # [S, half]
    cos = np.cos(freqs).T  # [half, S]
    sin = np.sin(freqs).T
    cosT = np.concatenate([cos, cos], axis=0).astype(dtype)  # [128, S]
    sinT = np.concatenate([sin, sin], axis=0).astype(dtype)
    return np.ascontiguousarray(cosT), np.ascontiguousarray(sinT)


def _rot_matrix(dtype=BF16):
    """Signed permutation R [128,128] (stationary layout) s.t.
    (R^T q)[i] = rotate_half(q)[i] for q in [d, s] layout."""
    half = HD // 2
    m = np.zeros((HD, HD), dtype=np.float32)
    for i in range(HD):
        m[(i + half) % HD, i] = -1.0 if i < half else 1.0
    return np.ascontiguousarray(m.astype(dtype))


def _prep_inputs(hidden_states, Wq, Wk, Wv, Wo, cfg, n_cores=N_CORES):
    """Build the per-core input dicts (all bf16)."""
    B, S, D, NH = cfg["B"], cfg["S"], cfg["D"], cfg["NH"]
    ET = D // 128
    DQ = NH * HD

    x = np.asarray(hidden_states, dtype=np.float32)
    xt = (
        np.ascontiguousarray(x.transpose(0, 2, 1))
        .astype(BF16)
        .reshape(B, ET, 128, S)
    )
    cosT, sinT = _rope_tables(S)
    rotmat = _rot_matrix()

    in_maps = []
    for c in range(n_cores):
        lo, hi = c * DQ, (c + 1) * DQ
        wq_c = np.ascontiguousarray(np.asarray(Wq)[lo:hi, :].T).astype(BF16)
        wk_c = np.ascontiguousarray(np.asarray(Wk)[lo:hi, :].T).astype(BF16)
        wv_c = np.ascontiguousarray(np.asarray(Wv)[lo:hi, :].T).astype(BF16)
        wo_c = np.ascontiguousarray(np.asarray(Wo)[:, lo:hi].T).astype(BF16)
        in_maps.append(
            {
                "xt": xt,
                "wq": wq_c.reshape(ET, 128, DQ),
                "wk": wk_c.reshape(ET, 128, DQ),
                "wv": wv_c.reshape(ET, 128, DQ),
                "wo": wo_c.reshape(NH, 128, D),
                "cos": cosT,
                "sin": sinT,
                "rotm": rotmat,
            }
        )
    return in_maps


def _gather(results, cfg):
    B, S, D = cfg["B"], cfg["S"], cfg["D"]
    acc = np.zeros((B, S, D), dtype=np.float64)
    for r in results:
        acc += np.asarray(r["out"]).astype(np.float64).reshape(B, S, D)
    return acc.astype(np.float32)


# ---------------------------------------------------------------- entry point
def kernel(hidden_states, Wq, Wk, Wv, Wo):
    from concourse.bass_utils import run_bass_kernel_spmd

    cfg = _full_cfg()
    key = ("nc", cfg["B"], cfg["S"], cfg["D"], cfg["NH"])
    if key not in _CACHE:
        _CACHE[key] = build_core_program(cfg["B"], cfg["S"], cfg["D"], cfg["NH"])
    nc = _CACHE[key]

    in_maps = _prep_inputs(hidden_states, Wq, Wk, Wv, Wo, cfg)
    res = run_bass_kernel_spmd(nc, in_maps, core_ids=list(range(N_CORES)))
    return _gather(res.results, cfg)
